# revision 15
# baseline (speedup 1.0000x reference)
"""InnerProductDecoder Trainium2 kernel.

adj = sigmoid(Zh @ Zh.T) per graph, Zh = Z @ W.T + b,
G=64 graphs x N=1024 nodes, D_IN=256, D_H=128.

Sharding: data-parallel over graphs, 8 graphs per NeuronCore on 8 cores.
W/b replicated. No collectives.

Per-core program (per graph g):
  1. Z_g [1024, 256] -> SBUF via gpsimd SWDGE (separate queue from the
     output stores so loads are never stuck behind store backlog).
  2. PE-transpose 128x128 blocks -> Z_g^T as 2 chunks [128d, 1024n].
  3. fc1: Zh^T[h, n] = W @ Z^T (+b): PSUM-accumulate over the 2 d-chunks,
     bias added during PSUM->SBUF eviction on DVE.
  4. S tiles: [128, 1024] PSUM (2 banks) = Zh^T[:, i].T @ Zh^T (h=128
     contraction), two 512-wide matmuls per tile.
  5. One 1024-wide sigmoid on ScalarE PSUM->SBUF per row tile; output
     stores batched 2 row tiles (1 MB) per dma_start on the sync ring.

Deep pools (zin=3, out=5) keep the store DMA stream (the 358 GB/s/core
HBM roofline: 40 MB/core -> ~112 us) busy end-to-end.
"""

import numpy as np

N_CORES = 8
G_PER_CORE = 8
N = 1024          # nodes per graph
D = 256           # input dim
H = 128           # hidden dim
NT = N // 128     # 128-row tiles per graph
JW = 512          # moving free dim for matmuls (fp32 max, 1 PSUM bank)
NJ = N // JW

# matmul input dtype knobs: "f32" (exact, 4 cyc/row) or "f32r" (1 cyc/row)
S_DTYPE = "f32r"
FC1_DTYPE = "f32r"

_CACHE = {}


def _build_nc():
    import concourse.bass as bass
    import concourse.tile as tile
    from concourse import bacc, masks, mybir
    from concourse._compat import get_trn_type

    f32 = mybir.dt.float32
    f32r = mybir.dt.float32r
    # Tensors consumed by an FP32r matmul must be *written* as f32r by their
    # producer (BIR verifier rule) — so matmul-input tiles take the mm dtype.
    fc1_dt = f32r if FC1_DTYPE == "f32r" else f32
    s_dt = f32r if S_DTYPE == "f32r" else f32

    nc = bacc.Bacc(get_trn_type() or "TRN2", target_bir_lowering=False, debug=False)
    Z_d = nc.declare_dram_parameter("Z", [G_PER_CORE * N, D], f32, isOutput=False)
    W_d = nc.declare_dram_parameter("W", [H, D], f32, isOutput=False)
    b_d = nc.declare_dram_parameter("b", [H, 1], f32, isOutput=False)
    adj_d = nc.declare_dram_parameter("adj", [G_PER_CORE * N, N], f32, isOutput=True)

    with tile.TileContext(nc) as tc:
        with (
            tc.tile_pool(name="consts", bufs=1) as consts,
            tc.tile_pool(name="zin", bufs=4) as zin_pool,
            tc.tile_pool(name="zt", bufs=2) as zt_pool,
            tc.tile_pool(name="zh", bufs=2) as zh_pool,
            tc.tile_pool(name="outp", bufs=5) as out_pool,
            tc.tile_pool(name="ps_tr", bufs=2, space=bass.MemorySpace.PSUM) as ps_tr,
            tc.tile_pool(name="ps_s", bufs=3, space=bass.MemorySpace.PSUM) as ps_s,
        ):
            ident = consts.tile([128, 128], f32)
            masks.make_identity(nc, ident[:])

            w_nat = consts.tile([128, D], f32)
            nc.sync.dma_start(w_nat[:], W_d[:])
            b_sb = consts.tile([128, 1], f32)
            nc.sync.dma_start(b_sb[:], b_d[:])

            # W^T as 2 chunks: wt[:, c, :] = W[:, c*128:(c+1)*128].T
            wt = consts.tile([128, 2, H], fc1_dt)
            for c in range(2):
                p = ps_tr.tile([128, 128], f32)
                nc.tensor.transpose(p[:], w_nat[:, c * 128:(c + 1) * 128], ident[:])
                nc.vector.tensor_copy(wt[:, c, :], p[:])

            # [g, p, t, d] view of Z: graph g, tile t, partition row p
            Zv = Z_d.rearrange("(g t p) d -> g p t d", g=G_PER_CORE, t=NT, p=128)
            # [g, p, t, n] view of adj for 2-tile batched stores
            Av = adj_d.rearrange("(g t p) n -> g p t n", g=G_PER_CORE, t=NT, p=128)

            # Z loads go on the scalar HWDGE ring (stores own the sync ring,
            # so loads never queue behind store backlog), in 256 KB chunks:
            # 16 descriptors per SDMA engine ~ 1-2 packets, so a chunk
            # completes in a couple of round-robin turns even under full
            # store pressure (a monolithic 1 MB load was observed
            # stretching to ~29 us). Chunk dispatches for graph g+2 are
            # interleaved between graph g's sigmoids so they never delay
            # them on the scalar sequencer.
            NCH = 4           # chunks per graph load
            CHT = NT // NCH   # 128-row tiles per chunk
            zins = {}

            def load_z_chunk(g, ch):
                if g >= G_PER_CORE:
                    return
                if ch == 0:
                    zins[g] = zin_pool.tile([128, NT, D], f32, name="zin")
                zin = zins[g]
                sl = slice(ch * CHT, (ch + 1) * CHT)
                nc.scalar.dma_start(zin[:, sl, :], Zv[g, :, sl, :])

            for g in (0, 1, 2):
                for ch in range(NCH):
                    load_z_chunk(g, ch)

            for g in range(G_PER_CORE):
                zin = zins.pop(g)

                # Z_g^T: zt[:, c, n] = Z_g[n, c*128 + d]
                zt = zt_pool.tile([128, 2, N], fc1_dt)
                for t in range(NT):
                    for c in range(2):
                        p = ps_tr.tile([128, 128], f32)
                        nc.tensor.transpose(
                            p[:], zin[:, t, c * 128:(c + 1) * 128], ident[:]
                        )
                        nc.vector.tensor_copy(zt[:, c, t * 128:(t + 1) * 128], p[:])

                # fc1: Zh^T [h, n] = W @ Z_g^T + b (both 512-chunks in one
                # 2-bank PSUM tile from the shared pool). Evict each 512
                # chunk right after its matmuls so the j=0 bias-add on DVE
                # overlaps the j=1 matmuls on PE (a single 1024-wide
                # eviction put ~1.4 us of DVE latency between fc1 and the
                # first S matmul on every graph).
                pf = ps_s.tile([128, N], f32, name="ps")
                zh = zh_pool.tile([128, N], s_dt)
                for j in range(NJ):
                    for c in range(2):
                        nc.tensor.matmul(
                            pf[:, j * JW:(j + 1) * JW],
                            wt[:, c, :],
                            zt[:, c, j * JW:(j + 1) * JW],
                            start=(c == 0),
                            stop=(c == 1),
                        )
                    nc.vector.tensor_scalar_add(
                        zh[:, j * JW:(j + 1) * JW],
                        pf[:, j * JW:(j + 1) * JW],
                        b_sb[:],
                    )

                # S = Zh @ Zh^T, sigmoid, store 2 row tiles per DMA
                for i2 in range(NT // 2):
                    load_z_chunk(g + 3, i2)
                    ot = out_pool.tile([128, 2, N], f32)
                    for k in range(2):
                        i = 2 * i2 + k
                        ps = ps_s.tile([128, N], f32, name="ps")
                        for j in range(NJ):
                            nc.tensor.matmul(
                                ps[:, j * JW:(j + 1) * JW],
                                zh[:, i * 128:(i + 1) * 128],
                                zh[:, j * JW:(j + 1) * JW],
                            )
                        nc.scalar.activation(
                            ot[:, k, :],
                            ps[:],
                            mybir.ActivationFunctionType.Sigmoid,
                        )
                    nc.sync.dma_start(Av[g, :, 2 * i2:2 * i2 + 2, :], ot[:])

    nc.compile()
    return nc


def _get_nc():
    if "nc" not in _CACHE:
        _CACHE["nc"] = _build_nc()
    return _CACHE["nc"]


def run(Z, W, b, trace=False):
    from concourse.bass_utils import run_bass_kernel_spmd

    Z = np.ascontiguousarray(np.asarray(Z, dtype=np.float32))
    W = np.ascontiguousarray(np.asarray(W, dtype=np.float32))
    b = np.ascontiguousarray(np.asarray(b, dtype=np.float32)).reshape(H, 1)
    assert Z.shape == (N_CORES * G_PER_CORE * N, D)

    nc = _get_nc()
    rows = G_PER_CORE * N
    in_maps = [
        {"Z": Z[c * rows:(c + 1) * rows], "W": W, "b": b} for c in range(N_CORES)
    ]
    res = run_bass_kernel_spmd(nc, in_maps, list(range(N_CORES)), trace=trace)
    out = np.concatenate([r["adj"] for r in res.results], axis=0)
    return out.reshape(N_CORES * G_PER_CORE, N, N), res


def kernel(Z=None, W=None, b=None, node_slice=None, **kwargs):
    out, _ = run(Z, W, b)
    return out
